# revision 9
# baseline (speedup 1.0000x reference)
"""Distributed flash-decoding attention kernel for 8 TRN2 NeuronCores.

Problem: B=1024 new tokens attend over a 32768-row KV cache plus the new
block (causal within the block). Sequence-parallel sharding: each core
handles 4096 cache rows + 128 new rows (4224 keys = 33 key tiles).

Per key tile t (all fp16 operands, f32 PSUM):
  scores s_t = kt_t^T @ qt            [128 keys, 1024 q]  (2 matmuls of 512)
  e_t = exp(s_t) -> fp16              ACT engine for 25 tiles; DVE computes
                                      a 2-phase Schraudolph exp2 approx
                                      (max rel err ~0.8%) for 8 tiles so
                                      exp never bottlenecks the ACT engine
  oa_qs += e_t[:, qs]^T @ vaug_t      [128 q, 132] x 8 q-subtiles, vaug has
                                      a ones column so the softmax
                                      normalizer falls out of the matmul
A bf16 [1024, 132] partial then goes through a ReduceScatter; each core
normalizes and emits its 128-query slice.
"""

import os
import sys

import numpy as np

for _p in ("/opt/trn_rl_repo",):
    if os.path.isdir(_p) and _p not in sys.path:
        sys.path.insert(0, _p)

import ml_dtypes  # noqa: E402
import concourse.bacc as bacc  # noqa: E402
import concourse.mybir as mybir  # noqa: E402
import concourse.tile as tile  # noqa: E402
from concourse.bass_utils import run_bass_kernel_spmd  # noqa: E402

N_CORES = 8
B, S, DK, DV = 1024, 32768, 128, 128
S_SH = S // N_CORES  # 4096 cache rows per core
B_SH = B // N_CORES  # 128 new rows per core
NKEY = S_SH + B_SH  # 4224 keys per core
NT = NKEY // 128  # 33 key tiles
DVA = DV + 4  # 132: dv cols + l col (128) + 3 pad
F32 = mybir.dt.float32
F16 = mybir.dt.float16
BF16 = mybir.dt.bfloat16
I16 = mybir.dt.int16
I32 = mybir.dt.int32

# key tiles whose exp runs on DVE (2-phase Schraudolph); rest on ACT.
DVE_TILES = frozenset((2, 6, 10, 14, 18, 22, 26, 30))
SCH_A = float(2.0**10 / np.log(2.0))  # fp16 exponent scale
SCH_C1 = float(15 * 2**10 - 2**10 - 58.0)  # folds the x0.5 of the average
SCH_C2 = SCH_C1 - 512.0  # half-period phase shift
SCH_W = 1.42  # tuned second-phase weight

KT_CHUNKS = [(0, 3), (3, 10), (13, 10), (23, 10)]  # (first tile, n tiles)
VA_CHUNKS = [(0, 3), (3, 15), (18, 15)]


def _declare_io(nc):
    return dict(
        kt=nc.dram_tensor("kt", [128, NKEY], F16, kind="ExternalInput"),
        qt=nc.dram_tensor("qt", [128, B], F16, kind="ExternalInput"),
        vaug=nc.dram_tensor("vaug", [NT, 128, DVA], F16, kind="ExternalInput"),
        thr=nc.dram_tensor("thr", [128, 1], F32, kind="ExternalInput"),
        out=nc.dram_tensor("out", [B_SH, DV], F32, kind="ExternalOutput"),
    )


def _emit_mask(nc, pmisc, th_d):
    """mask01[p, f] = 1.0 if query f >= (c*128 + p) else 0.0 (fp16)."""
    iota_i = pmisc.tile([128, B], I32, name="iota_i", tag="iota_i")
    nc.gpsimd.iota(iota_i[:], pattern=[[1, B]], base=0, channel_multiplier=0)
    iota_f = pmisc.tile([128, B], F32, name="iota_f", tag="iota_f")
    nc.vector.tensor_copy(iota_f[:], iota_i[:])
    thr_sb = pmisc.tile([128, 1], F32, name="thr", tag="thr")
    nc.sync.dma_start(thr_sb[:], th_d[:])
    mask01 = pmisc.tile([128, B], F16, name="mask", tag="mask")
    nc.vector.tensor_scalar(
        out=mask01[:],
        in0=iota_f[:],
        scalar1=thr_sb[:],
        scalar2=None,
        op0=mybir.AluOpType.is_ge,
    )
    return mask01


def _emit_body(nc, pools, io, mask01, part):
    """Loads + compute for one pass; writes the [B, DVA] bf16 partial."""
    pkt, pqt, pva, pexp = (
        pools["pkt"],
        pools["pqt"],
        pools["pva"],
        pools["pexp"],
    )
    ps_s, ps_oa = pools["ps_s"], pools["ps_oa"]

    # qt + kt stream on the SP HWDGE ring; vaug on the gpsimd ring.
    qt_sb = pqt.tile([128, B], F16, name="qt", tag="qt")
    nc.sync.dma_start(qt_sb[:], io["qt"][:])
    va_sbs = []  # (first_tile_idx, n_tiles, tile)
    fi, n = VA_CHUNKS[0]
    va_t = pva.tile([128, n, DVA], F16, name="va0", tag="va0")
    nc.gpsimd.dma_start(
        va_t[:], io["vaug"][fi : fi + n, :, :].rearrange("t p d -> p t d")
    )
    va_sbs.append((fi, n, va_t))
    kt_sbs = []  # (first tile, n tiles, tile)
    for i, (fi, n) in enumerate(KT_CHUNKS):
        t = pkt.tile([128, n * 128], F16, name=f"kt{i}", tag=f"kt{i}")
        nc.sync.dma_start(t[:], io["kt"][:, fi * 128 : (fi + n) * 128])
        kt_sbs.append((fi, n, t))
    for ci, (fi, n) in enumerate(VA_CHUNKS[1:], start=1):
        va_t = pva.tile([128, n, DVA], F16, name=f"va{ci}", tag=f"va{ci}")
        nc.gpsimd.dma_start(
            va_t[:], io["vaug"][fi : fi + n, :, :].rearrange("t p d -> p t d")
        )
        va_sbs.append((fi, n, va_t))

    def va_ap_for(t):
        for fi, n, tile_ in va_sbs:
            if fi <= t < fi + n:
                return tile_[:, t - fi, :]
        raise AssertionError(t)

    def kt_ap_for(t):
        for fi, n, tile_ in kt_sbs:
            if fi <= t < fi + n:
                return tile_[:, (t - fi) * 128 : (t - fi + 1) * 128]
        raise AssertionError(t)

    # PSUM: 2 banks per score tile (bufs=2 -> 4 banks) + 4 accumulator
    # banks. A start=True matmul marks its whole 2KB bank pending-zero, so
    # each accumulation group needs its own bank: PV runs in two passes of
    # 4 q-subtiles each, re-reading the resident e tiles in pass B.
    oaA = [
        ps_oa.tile([128, DVA], F32, name=f"oaA{qs}", tag=f"oa{qs}")
        for qs in range(4)
    ]

    es = []  # per-tile fp16 exp tiles

    def pv(t, qs_list, accs, last):
        va_ap = va_ap_for(t)
        for i, qs in enumerate(qs_list):
            nc.tensor.matmul(
                accs[i][:],
                es[t][:, qs * 128 : (qs + 1) * 128],
                va_ap,
                start=(t == 0),
                stop=last,
            )

    for t in range(NT):
        kt_ap = kt_ap_for(t)
        s_ps = ps_s.tile([128, B], F32, name="s", tag="s")
        for qh in range(2):
            nc.tensor.matmul(
                s_ps[:, qh * 512 : (qh + 1) * 512],
                kt_ap,
                qt_sb[:, qh * 512 : (qh + 1) * 512],
                start=True,
                stop=True,
            )
        if t in DVE_TILES:
            y1 = pexp.tile([128, B], I16, name="y1", tag="y1")
            nc.vector.tensor_scalar(
                out=y1[:],
                in0=s_ps[:],
                scalar1=SCH_A,
                scalar2=SCH_C1,
                op0=mybir.AluOpType.mult,
                op1=mybir.AluOpType.add,
            )
            y2 = pexp.tile([128, B], I16, name="y2", tag="y2")
            nc.vector.tensor_scalar(
                out=y2[:],
                in0=s_ps[:],
                scalar1=SCH_A,
                scalar2=SCH_C2,
                op0=mybir.AluOpType.mult,
                op1=mybir.AluOpType.add,
            )
            e_sb = pexp.tile([128, B], F16, name="e", tag="e", bufs=NT + 1)
            nc.vector.scalar_tensor_tensor(
                out=e_sb[:],
                in0=y2[:].bitcast(F16),
                scalar=SCH_W,
                in1=y1[:].bitcast(F16),
                op0=mybir.AluOpType.mult,
                op1=mybir.AluOpType.add,
            )
        else:
            e_sb = pexp.tile([128, B], F16, name="e", tag="e", bufs=NT + 1)
            nc.scalar.activation(
                e_sb[:], s_ps[:], mybir.ActivationFunctionType.Exp
            )
        if t == NT - 1:
            e_m = pexp.tile([128, B], F16, name="em", tag="em")
            nc.vector.tensor_tensor(
                out=e_m[:], in0=e_sb[:], in1=mask01[:], op=mybir.AluOpType.mult
            )
            e_sb = e_m
        es.append(e_sb)
        if t >= 1:
            pv(t - 1, range(4), oaA, last=False)
    pv(NT - 1, range(4), oaA, last=True)

    # pass A results: PSUM f32 -> SBUF bf16 (overlaps pass B), 1 DMA out.
    oa_sb = pexp.tile([128, 8, DVA], BF16, name="oa_sb", tag="oa_sb")
    for qs in range(4):
        if qs % 2 == 0:
            nc.vector.tensor_copy(oa_sb[:, qs, :], oaA[qs][:])
        else:
            nc.scalar.copy(oa_sb[:, qs, :], oaA[qs][:])
    nc.sync.dma_start(
        part[:].rearrange("(t p) d -> p t d", t=8)[:, 0:4, :],
        oa_sb[:, 0:4, :],
    )

    # pass B: PV for q-subtiles 4..7 re-reading the resident e tiles.
    oaB = [
        ps_oa.tile([128, DVA], F32, name=f"oaB{qs}", tag=f"oa{qs}")
        for qs in range(4)
    ]
    for t in range(NT):
        pv(t, (4, 5, 6, 7), oaB, last=(t == NT - 1))
    for qs in range(4):
        if qs % 2 == 0:
            nc.vector.tensor_copy(oa_sb[:, 4 + qs, :], oaB[qs][:])
        else:
            nc.scalar.copy(oa_sb[:, 4 + qs, :], oaB[qs][:])
    nc.sync.dma_start(
        part[:].rearrange("(t p) d -> p t d", t=8)[:, 4:8, :],
        oa_sb[:, 4:8, :],
    )


def _emit_combine(nc, pep, part, red, out_d):
    nc.gpsimd.collective_compute(
        "ReduceScatter",
        mybir.AluOpType.add,
        replica_groups=[list(range(N_CORES))],
        ins=[part.opt()],
        outs=[red.opt()],
    )
    red_sb = pep.tile([B_SH, DVA], BF16, name="red_sb", tag="red_sb")
    nc.sync.dma_start(red_sb[:], red[:])
    lf32 = pep.tile([B_SH, 1], F32, name="lf32", tag="lf32")
    nc.vector.tensor_copy(lf32[:], red_sb[:, DV : DV + 1])
    linv = pep.tile([B_SH, 1], F32, name="linv", tag="linv")
    nc.vector.reciprocal(linv[:], lf32[:])
    out_sb = pep.tile([B_SH, DV], F32, name="out_sb", tag="out_sb")
    nc.vector.tensor_scalar_mul(out_sb[:], red_sb[:, :DV], linv[:])
    nc.sync.dma_start(out_d[:], out_sb[:])


def build_nc(loop_iters: int | None = None, stage: int = 4):
    """loop_iters=None: real kernel (compute + ReduceScatter + epilogue).
    loop_iters=N: timing variant - compute body inside tc.For_i(0, N, 1),
    no collective (collectives can't sit inside control flow)."""
    nc = bacc.Bacc(
        "TRN2", target_bir_lowering=False, debug=False, num_devices=N_CORES
    )
    io = _declare_io(nc)
    with tile.TileContext(nc) as tc:
        with (
            tc.tile_pool(name="pkt", bufs=2) as pkt,
            tc.tile_pool(name="pqt", bufs=2) as pqt,
            tc.tile_pool(name="pva", bufs=2) as pva,
            tc.tile_pool(name="pexp", bufs=2) as pexp,
            tc.tile_pool(name="pmisc", bufs=1) as pmisc,
            tc.tile_pool(name="pep", bufs=2) as pep,
            tc.tile_pool(name="ps_s", bufs=2, space="PSUM") as ps_s,
            tc.tile_pool(name="ps_oa", bufs=1, space="PSUM") as ps_oa,
            tc.tile_pool(name="pdram", bufs=2, space="DRAM") as pdram,
        ):
            pools = dict(pkt=pkt, pqt=pqt, pva=pva, pexp=pexp, ps_s=ps_s, ps_oa=ps_oa)
            mask01 = _emit_mask(nc, pmisc, io["thr"])
            if loop_iters is None:
                part = pdram.tile([B, DVA], BF16, name="part", tag="part")
                red = pdram.tile([B_SH, DVA], BF16, name="red", tag="red")
                _emit_body(nc, pools, io, mask01, part)
                _emit_combine(nc, pep, part, red, io["out"])
            elif loop_iters == 0:
                part = pdram.tile([B, DVA], BF16, name="part", tag="part")
                _emit_body(nc, pools, io, mask01, part)
                out_sb = pep.tile([B_SH, DV], F32, name="out_sb0", tag="out_sb")
                nc.vector.memset(out_sb[:], 0.0)
                nc.sync.dma_start(io["out"][:], out_sb[:])
            else:
                part = pdram.tile([B, DVA], BF16, name="part", tag="part")
                with tc.For_i(0, loop_iters, 1):
                    _emit_body(nc, pools, io, mask01, part)
                out_sb = pep.tile([B_SH, DV], F32, name="out_sb", tag="out_sb")
                nc.vector.memset(out_sb[:], 0.0)
                nc.sync.dma_start(io["out"][:], out_sb[:])
    nc.compile()
    return nc


_CACHE: dict = {}


def _get_nc():
    if "nc" not in _CACHE:
        _CACHE["nc"] = build_nc()
    return _CACHE["nc"]


def make_in_maps(q, k, v, K_cache, V_cache):
    q = np.asarray(q, np.float32)
    k = np.asarray(k, np.float32)
    v = np.asarray(v, np.float32)
    K_cache = np.asarray(K_cache, np.float32)
    V_cache = np.asarray(V_cache, np.float32)

    scale = 1.0 / np.sqrt(np.float32(DK))
    qt = np.ascontiguousarray((q * scale).T).astype(np.float16)

    in_maps = []
    for c in range(N_CORES):
        Ksh = np.concatenate(
            [K_cache[c * S_SH : (c + 1) * S_SH], k[c * B_SH : (c + 1) * B_SH]],
            axis=0,
        )  # [4224, 128]
        kt = np.ascontiguousarray(Ksh.T).astype(np.float16)
        Vsh = np.concatenate(
            [V_cache[c * S_SH : (c + 1) * S_SH], v[c * B_SH : (c + 1) * B_SH]],
            axis=0,
        )
        va = np.zeros((NKEY, DVA), np.float32)
        va[:, :DV] = Vsh
        va[:, DV] = 1.0
        va = va.reshape(NT, 128, DVA).astype(np.float16)
        thr = (c * B_SH + np.arange(128, dtype=np.float32)).reshape(128, 1)
        in_maps.append({"kt": kt, "qt": qt, "vaug": va, "thr": thr})
    return in_maps


def kernel(q, k, v, K_cache, V_cache):
    in_maps = make_in_maps(q, k, v, K_cache, V_cache)
    res = run_bass_kernel_spmd(
        _get_nc(), in_maps, core_ids=list(range(N_CORES))
    )
    out = np.concatenate(
        [res.results[c]["out"] for c in range(N_CORES)], axis=0
    )
    return np.ascontiguousarray(out, dtype=np.float32)


# revision 15
# speedup vs baseline: 1.6394x; 1.6394x over previous
"""Distributed flash-decoding attention kernel for 8 TRN2 NeuronCores.

Problem: B=1024 new tokens attend over a 32768-row KV cache plus the new
block (causal within the block). Sequence-parallel sharding: each core
handles 4096 cache rows + 128 new rows (4224 keys = 33 key tiles).

Per key tile t (all fp16 operands, f32 PSUM):
  scores s_t = kt_t^T @ qt            [128 keys, 1024 q]  (2 matmuls of 512)
  e_t = exp(s_t) -> fp16              ACT engine for 25 tiles; DVE computes
                                      a 2-phase Schraudolph exp2 approx
                                      (max rel err ~0.8%) for 8 tiles so
                                      exp never bottlenecks the ACT engine
  oa_qs += e_t[:, qs]^T @ vaug_t      [128 q, 132] x 8 q-subtiles, vaug has
                                      a ones column so the softmax
                                      normalizer falls out of the matmul
A bf16 [1024, 132] partial then goes through a ReduceScatter; each core
normalizes and emits its 128-query slice.
"""

import os
import sys

import numpy as np

for _p in ("/opt/trn_rl_repo",):
    if os.path.isdir(_p) and _p not in sys.path:
        sys.path.insert(0, _p)

import ml_dtypes  # noqa: E402
import concourse.bacc as bacc  # noqa: E402
import concourse.mybir as mybir  # noqa: E402
import concourse.tile as tile  # noqa: E402
from concourse.bass_utils import run_bass_kernel_spmd  # noqa: E402

N_CORES = 8
B, S, DK, DV = 1024, 32768, 128, 128
S_SH = S // N_CORES  # 4096 cache rows per core
B_SH = B // N_CORES  # 128 new rows per core
NKEY = S_SH + B_SH  # 4224 keys per core
NT = NKEY // 128  # 33 key tiles
DVA = DV + 4  # 132: dv cols + l col (128) + 3 pad
F32 = mybir.dt.float32
F16 = mybir.dt.float16
BF16 = mybir.dt.bfloat16
I16 = mybir.dt.int16
I32 = mybir.dt.int32

# key tiles whose exp runs on DVE (2-phase Schraudolph); rest on ACT.
DVE_TILES = frozenset(range(1, 32, 3))
SCH_A = float(2.0**10 / np.log(2.0))  # fp16 exponent scale
SCH_C1 = float(15 * 2**10 - 2**10 - 58.0)  # folds the x0.5 of the average
SCH_C2 = SCH_C1 - 512.0  # half-period phase shift
SCH_W = 1.42  # tuned second-phase weight

KT_CHUNKS = [(0, 3), (3, 10), (13, 10), (23, 10)]  # (first tile, n tiles)
VA_CHUNKS = [(0, 3), (3, 15), (18, 15)]


def _declare_io(nc):
    return dict(
        kt=nc.dram_tensor("kt", [128, NKEY], F16, kind="ExternalInput"),
        qt=nc.dram_tensor("qt", [128, B], F16, kind="ExternalInput"),
        vaug=nc.dram_tensor("vaug", [NT, 128, DVA], F16, kind="ExternalInput"),
        thr=nc.dram_tensor("thr", [128, 1], F32, kind="ExternalInput"),
        out=nc.dram_tensor("out", [B_SH, DV], F32, kind="ExternalOutput"),
    )


def _emit_mask(nc, pmisc, th_d):
    """mask01[p, f] = 1.0 if query f >= (c*128 + p) else 0.0 (fp16)."""
    iota_i = pmisc.tile([128, B], I32, name="iota_i", tag="iota_i")
    nc.gpsimd.iota(iota_i[:], pattern=[[1, B]], base=0, channel_multiplier=0)
    iota_f = pmisc.tile([128, B], F32, name="iota_f", tag="iota_f")
    nc.vector.tensor_copy(iota_f[:], iota_i[:])
    thr_sb = pmisc.tile([128, 1], F32, name="thr", tag="thr")
    nc.sync.dma_start(thr_sb[:], th_d[:])
    mask01 = pmisc.tile([128, B], F16, name="mask", tag="mask")
    nc.vector.tensor_scalar(
        out=mask01[:],
        in0=iota_f[:],
        scalar1=thr_sb[:],
        scalar2=None,
        op0=mybir.AluOpType.is_ge,
    )
    return mask01


def _emit_body(nc, pools, io, mask01, part):
    """Loads + compute for one pass; writes the [B, DVA] bf16 partial."""
    pkt, pqt, pva, pexp = (
        pools["pkt"],
        pools["pqt"],
        pools["pva"],
        pools["pexp"],
    )
    ps_s, ps_oa = pools["ps_s"], pools["ps_oa"]

    # qt + kt stream on the SP HWDGE ring; vaug on the ACT HWDGE ring.
    qt_sb = pqt.tile([128, B], F16, name="qt", tag="qt")
    nc.sync.dma_start(qt_sb[:], io["qt"][:])
    va_sbs = []  # (first_tile_idx, n_tiles, tile)
    fi, n = VA_CHUNKS[0]
    va_t = pva.tile([128, n, DVA], F16, name="va0", tag="va0")
    nc.scalar.dma_start(
        va_t[:], io["vaug"][fi : fi + n, :, :].rearrange("t p d -> p t d")
    )
    va_sbs.append((fi, n, va_t))
    kt_sbs = []  # (first tile, n tiles, tile)
    for i, (fi, n) in enumerate(KT_CHUNKS):
        t = pkt.tile([128, n * 128], F16, name=f"kt{i}", tag=f"kt{i}")
        nc.sync.dma_start(t[:], io["kt"][:, fi * 128 : (fi + n) * 128])
        kt_sbs.append((fi, n, t))
    for ci, (fi, n) in enumerate(VA_CHUNKS[1:], start=1):
        va_t = pva.tile([128, n, DVA], F16, name=f"va{ci}", tag=f"va{ci}")
        nc.scalar.dma_start(
            va_t[:], io["vaug"][fi : fi + n, :, :].rearrange("t p d -> p t d")
        )
        va_sbs.append((fi, n, va_t))

    def va_ap_for(t):
        for fi, n, tile_ in va_sbs:
            if fi <= t < fi + n:
                return tile_[:, t - fi, :]
        raise AssertionError(t)

    def kt_ap_for(t):
        for fi, n, tile_ in kt_sbs:
            if fi <= t < fi + n:
                return tile_[:, (t - fi) * 128 : (t - fi + 1) * 128]
        raise AssertionError(t)

    # PSUM: 2 banks per score tile (bufs=2 -> 4 banks) + 4 accumulator
    # banks. A start=True matmul marks its whole 2KB bank pending-zero, so
    # each accumulation group needs its own bank: PV runs in two passes of
    # 4 q-subtiles each, re-reading the resident e tiles in pass B.
    oaA = [
        ps_oa.tile([128, DVA], F32, name=f"oaA{qs}", tag=f"oa{qs}")
        for qs in range(4)
    ]

    es = []  # per-tile fp16 exp tiles

    def pv(t, qs_list, accs, last):
        va_ap = va_ap_for(t)
        for i, qs in enumerate(qs_list):
            nc.tensor.matmul(
                accs[i][:],
                es[t][:, qs * 128 : (qs + 1) * 128],
                va_ap,
                start=(t == 0),
                stop=last,
            )

    for t in range(NT):
        kt_ap = kt_ap_for(t)
        s_ps = ps_s.tile([128, B], F32, name="s", tag="s")
        for qh in range(2):
            nc.tensor.matmul(
                s_ps[:, qh * 512 : (qh + 1) * 512],
                kt_ap,
                qt_sb[:, qh * 512 : (qh + 1) * 512],
                start=True,
                stop=True,
            )
        if t in DVE_TILES:
            y1 = pexp.tile([128, B], I16, name="y1", tag="y1")
            nc.vector.tensor_scalar(
                out=y1[:],
                in0=s_ps[:],
                scalar1=SCH_A,
                scalar2=SCH_C1,
                op0=mybir.AluOpType.mult,
                op1=mybir.AluOpType.add,
            )
            # y2 = y1 - 512 in int16 == round(s*A + C1 - 512): the halved
            # exponent-phase shift, computed without re-reading PSUM.
            y2 = pexp.tile([128, B], I16, name="y2", tag="y2")
            nc.vector.tensor_scalar(
                out=y2[:],
                in0=y1[:],
                scalar1=-512,
                scalar2=None,
                op0=mybir.AluOpType.add,
            )
            e_sb = pexp.tile([128, B], F16, name="e", tag="e", bufs=NT + 1)
            nc.vector.scalar_tensor_tensor(
                out=e_sb[:],
                in0=y2[:].bitcast(F16),
                scalar=SCH_W,
                in1=y1[:].bitcast(F16),
                op0=mybir.AluOpType.mult,
                op1=mybir.AluOpType.add,
            )
        else:
            e_sb = pexp.tile([128, B], F16, name="e", tag="e", bufs=NT + 1)
            nc.scalar.activation(
                e_sb[:], s_ps[:], mybir.ActivationFunctionType.Exp
            )
        if t == NT - 1:
            e_m = pexp.tile([128, B], F16, name="em", tag="em")
            nc.vector.tensor_tensor(
                out=e_m[:], in0=e_sb[:], in1=mask01[:], op=mybir.AluOpType.mult
            )
            e_sb = e_m
        es.append(e_sb)
        if t >= 1:
            pv(t - 1, range(4), oaA, last=False)
    pv(NT - 1, range(4), oaA, last=True)

    # pass A results: PSUM f32 -> SBUF f32 (overlaps pass B), 1 DMA out.
    oa_sb = pexp.tile([128, 8, DVA], F32, name="oa_sb", tag="oa_sb")
    for qs in range(4):
        nc.vector.tensor_copy(oa_sb[:, qs, :], oaA[qs][:])
    nc.sync.dma_start(
        part[:].rearrange("(t p) d -> p t d", t=8)[:, 0:4, :],
        oa_sb[:, 0:4, :],
    )

    # pass B: PV for q-subtiles 4..7 re-reading the resident e tiles.
    oaB = [
        ps_oa.tile([128, DVA], F32, name=f"oaB{qs}", tag=f"oa{qs}")
        for qs in range(4)
    ]
    for t in range(NT):
        pv(t, (4, 5, 6, 7), oaB, last=(t == NT - 1))
    for qs in range(4):
        nc.vector.tensor_copy(oa_sb[:, 4 + qs, :], oaB[qs][:])
    nc.sync.dma_start(
        part[:].rearrange("(t p) d -> p t d", t=8)[:, 4:8, :],
        oa_sb[:, 4:8, :],
    )


def _emit_combine(nc, pep, part, red, out_d):
    nc.gpsimd.collective_compute(
        "ReduceScatter",
        mybir.AluOpType.add,
        replica_groups=[list(range(N_CORES))],
        ins=[part.opt()],
        outs=[red.opt()],
    )
    red_sb = pep.tile([B_SH, DVA], F32, name="red_sb", tag="red_sb")
    nc.sync.dma_start(red_sb[:], red[:])
    linv = pep.tile([B_SH, 1], F32, name="linv", tag="linv")
    nc.vector.reciprocal(linv[:], red_sb[:, DV : DV + 1])
    out_sb = pep.tile([B_SH, DV], F32, name="out_sb", tag="out_sb")
    nc.vector.tensor_scalar_mul(out_sb[:], red_sb[:, :DV], linv[:])
    nc.sync.dma_start(out_d[:], out_sb[:])


def build_nc(loop_iters: int | None = None, stage: int = 4):
    """loop_iters=None: real kernel (compute + ReduceScatter + epilogue).
    loop_iters=N: timing variant - compute body inside tc.For_i(0, N, 1),
    no collective (collectives can't sit inside control flow)."""
    nc = bacc.Bacc(
        "TRN2", target_bir_lowering=False, debug=False, num_devices=N_CORES
    )
    io = _declare_io(nc)
    with tile.TileContext(nc) as tc:
        with (
            tc.tile_pool(name="pkt", bufs=2) as pkt,
            tc.tile_pool(name="pqt", bufs=2) as pqt,
            tc.tile_pool(name="pva", bufs=2) as pva,
            tc.tile_pool(name="pexp", bufs=2) as pexp,
            tc.tile_pool(name="pmisc", bufs=1) as pmisc,
            tc.tile_pool(name="pep", bufs=2) as pep,
            tc.tile_pool(name="ps_s", bufs=2, space="PSUM") as ps_s,
            tc.tile_pool(name="ps_oa", bufs=1, space="PSUM") as ps_oa,
            tc.tile_pool(name="pdram", bufs=2, space="DRAM") as pdram,
        ):
            pools = dict(pkt=pkt, pqt=pqt, pva=pva, pexp=pexp, ps_s=ps_s, ps_oa=ps_oa)
            mask01 = _emit_mask(nc, pmisc, io["thr"])
            if loop_iters is None:
                part = pdram.tile([B, DVA], F32, name="part", tag="part")
                red = pdram.tile([B_SH, DVA], F32, name="red", tag="red")
                _emit_body(nc, pools, io, mask01, part)
                _emit_combine(nc, pep, part, red, io["out"])
            elif loop_iters == 0:
                part = pdram.tile([B, DVA], F32, name="part", tag="part")
                _emit_body(nc, pools, io, mask01, part)
                out_sb = pep.tile([B_SH, DV], F32, name="out_sb0", tag="out_sb")
                nc.vector.memset(out_sb[:], 0.0)
                nc.sync.dma_start(io["out"][:], out_sb[:])
            else:
                part = pdram.tile([B, DVA], F32, name="part", tag="part")
                with tc.For_i(0, loop_iters, 1):
                    _emit_body(nc, pools, io, mask01, part)
                out_sb = pep.tile([B_SH, DV], F32, name="out_sb", tag="out_sb")
                nc.vector.memset(out_sb[:], 0.0)
                nc.sync.dma_start(io["out"][:], out_sb[:])
    nc.compile()
    return nc


_CACHE: dict = {}


def _get_nc():
    if "nc" not in _CACHE:
        _CACHE["nc"] = build_nc()
    return _CACHE["nc"]


def make_in_maps(q, k, v, K_cache, V_cache):
    q = np.asarray(q, np.float32)
    k = np.asarray(k, np.float32)
    v = np.asarray(v, np.float32)
    K_cache = np.asarray(K_cache, np.float32)
    V_cache = np.asarray(V_cache, np.float32)

    scale = 1.0 / np.sqrt(np.float32(DK))
    qt = np.ascontiguousarray((q * scale).T).astype(np.float16)

    in_maps = []
    for c in range(N_CORES):
        Ksh = np.concatenate(
            [K_cache[c * S_SH : (c + 1) * S_SH], k[c * B_SH : (c + 1) * B_SH]],
            axis=0,
        )  # [4224, 128]
        kt = np.ascontiguousarray(Ksh.T).astype(np.float16)
        Vsh = np.concatenate(
            [V_cache[c * S_SH : (c + 1) * S_SH], v[c * B_SH : (c + 1) * B_SH]],
            axis=0,
        )
        va = np.zeros((NKEY, DVA), np.float32)
        va[:, :DV] = Vsh
        va[:, DV] = 1.0
        va = va.reshape(NT, 128, DVA).astype(np.float16)
        thr = (c * B_SH + np.arange(128, dtype=np.float32)).reshape(128, 1)
        in_maps.append({"kt": kt, "qt": qt, "vaug": va, "thr": thr})
    return in_maps


def kernel(q, k, v, K_cache, V_cache):
    in_maps = make_in_maps(q, k, v, K_cache, V_cache)
    res = run_bass_kernel_spmd(
        _get_nc(), in_maps, core_ids=list(range(N_CORES))
    )
    out = np.concatenate(
        [res.results[c]["out"] for c in range(N_CORES)], axis=0
    )
    return np.ascontiguousarray(out, dtype=np.float32)
